# revision 22
# baseline (speedup 1.0000x reference)
"""Multi-head attention Trainium2 Bass kernel (8 NeuronCores).

Problem: B=2, S=2048, HIDDEN=1024, HEADS=16, HEAD=64 (torch-style MHA with
query-row masking).

Sharding: core c = (batch b = c//4, head-group g = c%4); each core owns 4
heads (256 Q/K/V features, column-split) of one batch and computes a
row-split partial of the output projection; the host sums the 4 partials
per batch.

Device-side dataflow is entirely in "feature-on-partition" transposed
layout so no on-chip transposes are needed:
  - host supplies x^T (hidden-major) activations with an augmented ones/mask
    row so biases ride inside the matmul,
  - scores are computed transposed S_T[k,q] per head; exp() without
    max-subtraction (scores are O(3)); softmax denominator comes from an
    appended ones-column on V (row 64 of the PV accumulator),
  - masked query rows are never sent to the device: softmax for a masked row
    is uniform, so its output is mean(V) @ Wo^T + bo, computed exactly on
    host. The device processes only packed unmasked query columns (padded to
    a fixed NQ for static shapes).

All matmuls run as float32r (full PE rate at free-dim >= 256). Scores for
several key-tiles are packed into one multi-bank PSUM tile so a single
ACTIVATE(Exp) covers them, amortizing the ~352-cycle ACT fixed cost.
"""

import os
import sys

# The Bass execute path runs through jax/PJRT on the axon-tunneled neuron
# cores; a JAX_PLATFORMS=cpu pin (used when running the jax reference) would
# hide them.
if os.environ.get("JAX_PLATFORMS") == "cpu":
    os.environ["JAX_PLATFORMS"] = ""

for _p in ("/opt/trn_rl_repo", "/root/.axon_site/_ro/trn_rl_repo"):
    if os.path.isdir(_p) and _p not in sys.path:
        sys.path.append(_p)

import numpy as np

HIDDEN = 1024
HEADS = 16
HEAD = 64
B, S = 2, 2048
NCORES = 8
GROUPS = 4             # head-groups (cores per batch)
DQ = HIDDEN // GROUPS  # per-core projected features = 4 heads * 64 = 256
NH = DQ // HEAD        # heads per core = 4
KAUG = HIDDEN + 1      # hidden + bias/mask row
NQ_PACKED = 1056       # padded unmasked-query columns = 4*256 + 32 tail

_cache = {}


def _qchunks(nq):
    if nq == NQ_PACKED:
        return [256, 256, 256, 256, 32]
    assert nq % 512 == 0
    return [512] * (nq // 512)


def _pchunks(nq):
    """Projection-phase free-dim chunks (<= 8 concurrent PSUM tiles)."""
    out = []
    rem = nq
    while rem > 0:
        c = min(512, rem)
        out.append(c)
        rem -= c
    return out


def _build(nq):
    """Build the Bass program for one core with nq packed query columns."""
    import concourse.mybir as mybir
    import concourse.tile as tile
    from concourse import bacc
    from concourse.bass import ts

    f32 = mybir.dt.float32
    f32r = mybir.dt.float32r
    Exp = mybir.ActivationFunctionType.Exp

    nc = bacc.Bacc()
    xq = nc.dram_tensor("xq", [KAUG, nq], f32, kind="ExternalInput")
    xk = nc.dram_tensor("xk", [KAUG, S], f32, kind="ExternalInput")
    xv = nc.dram_tensor("xv", [KAUG, S], f32, kind="ExternalInput")
    wq = nc.dram_tensor("wq", [KAUG, DQ], f32, kind="ExternalInput")
    wk = nc.dram_tensor("wk", [KAUG, DQ], f32, kind="ExternalInput")
    wv = nc.dram_tensor("wv", [KAUG, DQ], f32, kind="ExternalInput")
    wo = nc.dram_tensor("wo", [DQ, HIDDEN], f32, kind="ExternalInput")
    outp = nc.dram_tensor("outp", [HIDDEN, nq], f32, kind="ExternalOutput")

    KT = KAUG // 128          # 8 full k-tiles (tile 8 is the single aug row)
    ST = S // 128             # 16 key-position tiles
    MT = DQ // 128            # 2 M-tiles for the projections
    OT = HIDDEN // 128        # 8 out-feature tiles
    qcs = _qchunks(nq)
    qco = [sum(qcs[:i]) for i in range(len(qcs))]
    pcs = _pchunks(nq)
    pco = [sum(pcs[:i]) for i in range(len(pcs))]
    EXP_BUFS = 9 if nq <= NQ_PACKED else 4
    SCH = 512                 # free-dim chunk for the k projection

    with tile.TileContext(nc) as tc:
        with (
            tc.tile_pool(name="w", bufs=1) as w_pool,
            tc.tile_pool(name="persist", bufs=1) as persist,
            tc.tile_pool(name="exp", bufs=EXP_BUFS) as exp_pool,
            tc.tile_pool(name="nrm", bufs=4) as nrm_pool,
        ):
            # ---- weights, resident for the whole kernel (DMAs emitted at
            # first-use points so the serial DMA stream matches compute order)
            wq_sb = w_pool.tile([128, KT + 1, DQ], f32r, tag="wq")
            wk_sb = w_pool.tile([128, KT + 1, DQ], f32r, tag="wk")
            wv_sb = w_pool.tile([128, KT + 1, DQ], f32r, tag="wv")
            wo_sb = w_pool.tile([128, MT, HIDDEN], f32r, tag="wo")

            def dma_w(w_sb, w_h):
                nc.sync.dma_start(
                    out=w_sb[:, 0:KT, :],
                    in_=w_h[0:HIDDEN, :]
                    .rearrange("(t p) m -> p t m", p=128)
                    .bitcast(f32r),
                )
                nc.sync.dma_start(
                    out=w_sb[0:1, KT, :],
                    in_=w_h[HIDDEN : HIDDEN + 1, :].bitcast(f32r),
                )

            ones_f = w_pool.tile([128, ST * NH], f32, tag="onesf")
            nc.vector.memset(ones_f, 1.0)

            # ---- persistent activations ----
            qT_sb = persist.tile([128, MT, nq], f32r, tag="qT")  # [2 heads x 64, mt, q]
            kT_sb = persist.tile([128, MT, S], f32r, tag="kT")
            v_sb = persist.tile([128, ST, NH, HEAD + 1], f32r, tag="v")
            ctx_sb = persist.tile([128, MT, nq], f32r, tag="ctx")
            # ones column for the softmax denominators (row HEAD of PV output)
            nc.vector.tensor_copy(
                v_sb[:, :, :, HEAD : HEAD + 1],
                ones_f.rearrange("p (a b c) -> p a b c", a=ST, b=NH),
            )

            # ================= phase A: projections =================
            with (
                tc.tile_pool(name="xbuf", bufs=10) as x_pool,
                tc.tile_pool(name="proj_ps", bufs=8, space="PSUM") as proj_ps,
            ):
                # ---- Q projection: qT[dq, q] accumulated over 9 k-tiles ----
                xq_t = []
                for t in range(KT + 1):
                    pt = 128 if t < KT else 1
                    xt = x_pool.tile([128, nq], f32r, tag="x", name=f"xq_t{t}")
                    nc.sync.dma_start(
                        out=xt[:pt, :], in_=xq[t * 128 : t * 128 + pt, :].bitcast(f32r)
                    )
                    xq_t.append(xt)
                qps = {}
                for mi in range(MT):
                    for j in range(len(pcs)):
                        qps[mi, j] = proj_ps.tile(
                            [128, pcs[j]], f32, tag="ps", name=f"qps_{mi}_{j}"
                        )
                for t in range(KT + 1):
                    pt = 128 if t < KT else 1
                    for mi in range(MT):
                        for j in range(len(pcs)):
                            nc.tensor.matmul(
                                qps[mi, j],
                                wq_sb[:pt, t, ts(mi, 128)],
                                xq_t[t][:pt, pco[j] : pco[j] + pcs[j]],
                                start=(t == 0),
                                stop=(t == KT),
                            )
                for mi in range(MT):
                    for j in range(len(pcs)):
                        nc.vector.tensor_copy(
                            qT_sb[:, mi, pco[j] : pco[j] + pcs[j]], qps[mi, j]
                        )

                # ---- K projection: kT[dk, s] ----
                xk_t = []
                for t in range(KT + 1):
                    pt = 128 if t < KT else 1
                    xt = x_pool.tile([128, S], f32r, tag="x", name=f"xk_t{t}")
                    nc.sync.dma_start(
                        out=xt[:pt, :], in_=xk[t * 128 : t * 128 + pt, :].bitcast(f32r)
                    )
                    xk_t.append(xt)
                kps = {}
                for mi in range(MT):
                    for j in range(S // SCH):
                        kps[mi, j] = proj_ps.tile(
                            [128, SCH], f32, tag="ps", name=f"kps_{mi}_{j}"
                        )
                for t in range(KT + 1):
                    pt = 128 if t < KT else 1
                    for mi in range(MT):
                        for j in range(S // SCH):
                            nc.tensor.matmul(
                                kps[mi, j],
                                wk_sb[:pt, t, ts(mi, 128)],
                                xk_t[t][:pt, ts(j, SCH)],
                                start=(t == 0),
                                stop=(t == KT),
                            )
                for mi in range(MT):
                    for j in range(S // SCH):
                        nc.vector.tensor_copy(kT_sb[:, mi, ts(j, SCH)], kps[mi, j])

                # ---- V projection: v[s, dv] (natural layout, K-contiguous) ----
                xv_t = []
                for t in range(KT + 1):
                    pt = 128 if t < KT else 1
                    xt = x_pool.tile([128, S], f32r, tag="x", name=f"xv_t{t}")
                    nc.sync.dma_start(
                        out=xt[:pt, :], in_=xv[t * 128 : t * 128 + pt, :].bitcast(f32r)
                    )
                    xv_t.append(xt)
                for si in range(ST):
                    vps = proj_ps.tile([128, DQ], f32, tag="ps", name=f"vps_{si}")
                    for t in range(KT + 1):
                        pt = 128 if t < KT else 1
                        nc.tensor.matmul(
                            vps,
                            xv_t[t][:pt, ts(si, 128)],
                            wv_sb[:pt, t, :],
                            start=(t == 0),
                            stop=(t == KT),
                        )
                    nc.vector.tensor_copy(
                        v_sb[:, si, :, 0:HEAD],
                        vps.rearrange("p (h d) -> p h d", h=NH),
                    )

            # ============ phases B+C: attention + out-projection ============
            with (
                tc.tile_pool(name="exp", bufs=EXP_BUFS) as exp_pool,
                tc.tile_pool(name="nrm", bufs=4) as nrm_pool,
                tc.tile_pool(name="ostage", bufs=2) as out_pool,
                tc.tile_pool(name="sc_ps", bufs=2, space="PSUM") as sc_ps,
                tc.tile_pool(name="ctx_ps", bufs=2, space="PSUM") as ctx_ps,
                tc.tile_pool(name="o_ps", bufs=2, space="PSUM") as o_ps,
            ):
                for j, w in (enumerate(qcs) if 'b' in _PHASES else []):
                    qsl = slice(qco[j], qco[j] + w)
                    # kt group size / scores bank packing for this chunk width
                    if w <= 32:
                        grp, bcol = ST, w        # all 16 kt in one bank
                    elif w <= 256:
                        grp, bcol = 4, 256       # 4 kt across 2 banks
                    else:
                        grp, bcol = 2, 512       # 2 kt across 2 banks
                    for h in range(NH):
                        p0 = HEAD * (h % 2)
                        mt = h // 2
                        cps = ctx_ps.tile([HEAD + 1, w], f32, tag="ctx", name=f"c{j}_{h}")
                        for g in range(ST // grp):
                            scp = sc_ps.tile(
                                [128, grp, bcol], f32, tag="sc", name=f"s{j}_{h}_{g}"
                            )
                            for i in range(grp):
                                nc.tensor.matmul(
                                    scp[:, i, 0:w],
                                    kT_sb[p0 : p0 + HEAD, mt, ts(g * grp + i, 128)],
                                    qT_sb[p0 : p0 + HEAD, mt, qsl],
                                    start=True,
                                    stop=True,
                                )
                            ex = exp_pool.tile(
                                [128, grp, w], f32r, tag="exp", name=f"e{j}_{h}_{g}"
                            )
                            nc.scalar.activation(ex, scp[:, :, 0:w], Exp)
                            for i in range(grp):
                                kt = g * grp + i
                                nc.tensor.matmul(
                                    cps,
                                    v_sb[:, kt, h, :],
                                    ex[:, i, :],
                                    start=(kt == 0),
                                    stop=(kt == ST - 1),
                                )
                        rec = nrm_pool.tile([1, w], f32, tag="recip", name=f"r{j}_{h}")
                        nc.vector.reciprocal(rec, cps[HEAD : HEAD + 1, :])
                        bc = nrm_pool.tile([HEAD, w], f32, tag="bc", name=f"b{j}_{h}")
                        nc.gpsimd.partition_broadcast(bc, rec)
                        nc.vector.tensor_mul(
                            ctx_sb[p0 : p0 + HEAD, mt, qsl], cps[0:HEAD, :], bc
                        )

                    if 'c' not in _PHASES:
                        continue
                    # out-projection for this q-chunk
                    ost = out_pool.tile([128, OT, w], f32, tag="ost", name=f"o{j}")
                    for mi in range(OT):
                        op = o_ps.tile([128, w], f32, tag="ops", name=f"op{j}_{mi}")
                        for t2 in range(MT):
                            nc.tensor.matmul(
                                op,
                                wo_sb[:, t2, ts(mi, 128)],
                                ctx_sb[:, t2, qsl],
                                start=(t2 == 0),
                                stop=(t2 == MT - 1),
                            )
                        nc.vector.tensor_copy(ost[:, mi, :], op)
                    nc.sync.dma_start(
                        out=outp.rearrange("(t p) n -> p t n", p=128)[:, :, qsl],
                        in_=ost,
                    )

    nc.finalize()
    return nc


def _get_program(nq):
    if nq not in _cache:
        _cache[nq] = _build(nq)
    return _cache[nq]


def kernel(query, key, value, mask, Wq, bq, Wk, bk, Wv, bv, Wo, bo):
    from concourse.bass_utils import run_bass_kernel_spmd

    query = np.asarray(query, dtype=np.float32)
    key = np.asarray(key, dtype=np.float32)
    value = np.asarray(value, dtype=np.float32)
    mask = np.asarray(mask)
    Wq = np.asarray(Wq, dtype=np.float32)
    bq = np.asarray(bq, dtype=np.float32)
    Wk = np.asarray(Wk, dtype=np.float32)
    bk = np.asarray(bk, dtype=np.float32)
    Wv = np.asarray(Wv, dtype=np.float32)
    bv = np.asarray(bv, dtype=np.float32)
    Wo = np.asarray(Wo, dtype=np.float32)
    bo = np.asarray(bo, dtype=np.float32)

    idxs = [np.nonzero(mask[b] != 0)[0] for b in range(B)]
    packed = all(len(ix) <= NQ_PACKED for ix in idxs)
    if packed:
        nq = NQ_PACKED
    else:
        # Degenerate mask (can't happen for the reference seed): process every
        # query column; masked columns are zeroed + aug-row 0, which yields the
        # exact uniform-softmax rows on device.
        nq = S
        idxs = [np.arange(S) for _ in range(B)]

    scale = 1.0 / np.sqrt(np.float32(HEAD))
    in_maps = []
    for c in range(NCORES):
        b, g = divmod(c, GROUPS)
        rows = slice(DQ * g, DQ * (g + 1))

        ix = idxs[b]
        xq_h = np.zeros((KAUG, nq), np.float32)
        xcols = query[b][ix].T
        if not packed:
            xcols = xcols * (mask[b] != 0)
        xq_h[:HIDDEN, : len(ix)] = xcols
        xq_h[HIDDEN, : len(ix)] = (mask[b][ix] != 0).astype(np.float32)

        xk_h = np.empty((KAUG, S), np.float32)
        xk_h[:HIDDEN] = key[b].T
        xk_h[HIDDEN] = 1.0
        xv_h = np.empty((KAUG, S), np.float32)
        xv_h[:HIDDEN] = value[b].T
        xv_h[HIDDEN] = 1.0

        wq_h = np.empty((KAUG, DQ), np.float32)
        wq_h[:HIDDEN] = Wq[rows].T * scale
        wq_h[HIDDEN] = bq[rows] * scale
        wk_h = np.empty((KAUG, DQ), np.float32)
        wk_h[:HIDDEN] = Wk[rows].T
        wk_h[HIDDEN] = bk[rows]
        wv_h = np.empty((KAUG, DQ), np.float32)
        wv_h[:HIDDEN] = Wv[rows].T
        wv_h[HIDDEN] = bv[rows]
        wo_h = np.ascontiguousarray(Wo[:, rows].T)

        in_maps.append(
            {
                "xq": np.ascontiguousarray(xq_h),
                "xk": xk_h,
                "xv": xv_h,
                "wq": wq_h,
                "wk": wk_h,
                "wv": wv_h,
                "wo": wo_h,
            }
        )

    nc = _get_program(nq)
    res = run_bass_kernel_spmd(nc, in_maps, core_ids=list(range(NCORES)))

    out = np.empty((B, S, HIDDEN), np.float32)
    for b in range(B):
        part = sum(res.results[b * GROUPS + g]["outp"] for g in range(GROUPS))
        ix = idxs[b]
        out[b][ix] = part[:, : len(ix)].T + bo
        if packed:
            # masked rows: softmax is uniform -> mean(V) @ Wo^T + bo, exact.
            vbar = value[b].mean(axis=0) @ Wv.T + bv
            out[b][mask[b] == 0] = vbar @ Wo.T + bo
    return out


# revision 24
# speedup vs baseline: 1.1781x; 1.1781x over previous
"""Multi-head attention Trainium2 Bass kernel (8 NeuronCores).

Problem: B=2, S=2048, HIDDEN=1024, HEADS=16, HEAD=64 (torch-style MHA with
query-row masking).

Sharding: core c = (batch b = c//4, head-group g = c%4); each core owns 4
heads (256 Q/K/V features, column-split) of one batch and computes a
row-split partial of the output projection; the host sums the 4 partials
per batch.

Device-side dataflow is entirely in "feature-on-partition" transposed
layout so no on-chip transposes are needed:
  - host supplies x^T (hidden-major) activations with an augmented ones/mask
    row so biases ride inside the matmul,
  - scores are computed transposed S_T[k,q] per head; exp() without
    max-subtraction (scores are O(3)); softmax denominator comes from an
    appended ones-column on V (row 64 of the PV accumulator),
  - masked query rows are never sent to the device: softmax for a masked row
    is uniform, so its output is mean(V) @ Wo^T + bo, computed exactly on
    host. The device processes only packed unmasked query columns (padded to
    a fixed NQ for static shapes).

All matmuls run as float32r (full PE rate at free-dim >= 256). Scores for
several key-tiles are packed into one multi-bank PSUM tile so a single
ACTIVATE(Exp) covers them, amortizing the ~352-cycle ACT fixed cost.
"""

import os
import sys

# The Bass execute path runs through jax/PJRT on the axon-tunneled neuron
# cores; a JAX_PLATFORMS=cpu pin (used when running the jax reference) would
# hide them.
if os.environ.get("JAX_PLATFORMS") == "cpu":
    os.environ["JAX_PLATFORMS"] = ""

for _p in ("/opt/trn_rl_repo", "/root/.axon_site/_ro/trn_rl_repo"):
    if os.path.isdir(_p) and _p not in sys.path:
        sys.path.append(_p)

import numpy as np

HIDDEN = 1024
HEADS = 16
HEAD = 64
B, S = 2, 2048
NCORES = 8
GROUPS = 4             # head-groups (cores per batch)
DQ = HIDDEN // GROUPS  # per-core projected features = 4 heads * 64 = 256
NH = DQ // HEAD        # heads per core = 4
KAUG = HIDDEN + 1      # hidden + bias/mask row
NQ_PACKED = 1056       # padded unmasked-query columns = 4*256 + 32 tail

_cache = {}


def _qchunks(nq):
    if nq == NQ_PACKED:
        return [256, 256, 256, 256, 32]
    assert nq % 512 == 0
    return [512] * (nq // 512)


def _pchunks(nq):
    """Projection-phase free-dim chunks (<= 8 concurrent PSUM tiles)."""
    out = []
    rem = nq
    while rem > 0:
        c = min(512, rem)
        out.append(c)
        rem -= c
    return out


def _build(nq):
    """Build the Bass program for one core with nq packed query columns."""
    import concourse.mybir as mybir
    import concourse.tile as tile
    from concourse import bacc
    from concourse.bass import ts

    f32 = mybir.dt.float32
    f32r = mybir.dt.float32r
    f16 = mybir.dt.float16
    Exp = mybir.ActivationFunctionType.Exp

    nc = bacc.Bacc()
    xq = nc.dram_tensor("xq", [KAUG, nq], f16, kind="ExternalInput")
    xk = nc.dram_tensor("xk", [KAUG, S], f16, kind="ExternalInput")
    xv = nc.dram_tensor("xv", [KAUG, S], f16, kind="ExternalInput")
    wq = nc.dram_tensor("wq", [KAUG, DQ], f16, kind="ExternalInput")
    wk = nc.dram_tensor("wk", [KAUG, DQ], f16, kind="ExternalInput")
    wv = nc.dram_tensor("wv", [KAUG, DQ], f16, kind="ExternalInput")
    wo = nc.dram_tensor("wo", [DQ, HIDDEN], f16, kind="ExternalInput")
    outp = nc.dram_tensor("outp", [HIDDEN, nq], f32, kind="ExternalOutput")

    KT = KAUG // 128          # 8 full k-tiles (tile 8 is the single aug row)
    ST = S // 128             # 16 key-position tiles
    MT = DQ // 128            # 2 M-tiles for the projections
    OT = HIDDEN // 128        # 8 out-feature tiles
    qcs = _qchunks(nq)
    qco = [sum(qcs[:i]) for i in range(len(qcs))]
    pcs = _pchunks(nq)
    pco = [sum(pcs[:i]) for i in range(len(pcs))]
    EXP_BUFS = 9 if nq <= NQ_PACKED else 4
    SCH = 512                 # free-dim chunk for the k projection

    with tile.TileContext(nc) as tc:
        with (
            tc.tile_pool(name="w", bufs=1) as w_pool,
            tc.tile_pool(name="persist", bufs=1) as persist,
            tc.tile_pool(name="exp", bufs=EXP_BUFS) as exp_pool,
            tc.tile_pool(name="nrm", bufs=4) as nrm_pool,
        ):
            # ---- weights, resident for the whole kernel (DMAs emitted at
            # first-use points so the serial DMA stream matches compute order)
            wq_sb = w_pool.tile([128, KT + 1, DQ], f16, tag="wq")
            wk_sb = w_pool.tile([128, KT + 1, DQ], f16, tag="wk")
            wv_sb = w_pool.tile([128, KT + 1, DQ], f16, tag="wv")
            wo_sb = w_pool.tile([128, MT, HIDDEN], f16, tag="wo")

            def dma_w(w_sb, w_h):
                nc.sync.dma_start(
                    out=w_sb[:, 0:KT, :],
                    in_=w_h[0:HIDDEN, :]
                    .rearrange("(t p) m -> p t m", p=128)
                    ,
                )
                nc.sync.dma_start(
                    out=w_sb[0:1, KT, :],
                    in_=w_h[HIDDEN : HIDDEN + 1, :],
                )

            ones_f = w_pool.tile([128, ST * NH], f32, tag="onesf")
            nc.vector.memset(ones_f, 1.0)

            # ---- persistent activations ----
            qT_sb = persist.tile([128, MT, nq], f16, tag="qT")  # [2 heads x 64, mt, q]
            kT_sb = persist.tile([128, MT, S], f16, tag="kT")
            v_sb = persist.tile([128, ST, NH, HEAD + 1], f16, tag="v")
            ctx_sb = persist.tile([128, MT, nq], f16, tag="ctx")
            # ones column for the softmax denominators (row HEAD of PV output)
            nc.vector.tensor_copy(
                v_sb[:, :, :, HEAD : HEAD + 1],
                ones_f.rearrange("p (a b c) -> p a b c", a=ST, b=NH),
            )

            # ================= phase A: projections =================
            with (
                tc.tile_pool(name="xbuf", bufs=10) as x_pool,
                tc.tile_pool(name="proj_ps", bufs=8, space="PSUM") as proj_ps,
            ):
                # ---- Q projection: qT[dq, q] accumulated over 9 k-tiles ----
                xq_t = []
                for t in range(KT + 1):
                    pt = 128 if t < KT else 1
                    xt = x_pool.tile([128, nq], f16, tag="x", name=f"xq_t{t}")
                    nc.sync.dma_start(
                        out=xt[:pt, :], in_=xq[t * 128 : t * 128 + pt, :]
                    )
                    xq_t.append(xt)
                qps = {}
                for mi in range(MT):
                    for j in range(len(pcs)):
                        qps[mi, j] = proj_ps.tile(
                            [128, pcs[j]], f32, tag="ps", name=f"qps_{mi}_{j}"
                        )
                for t in range(KT + 1):
                    pt = 128 if t < KT else 1
                    for mi in range(MT):
                        for j in range(len(pcs)):
                            nc.tensor.matmul(
                                qps[mi, j],
                                wq_sb[:pt, t, ts(mi, 128)],
                                xq_t[t][:pt, pco[j] : pco[j] + pcs[j]],
                                start=(t == 0),
                                stop=(t == KT),
                            )
                for mi in range(MT):
                    for j in range(len(pcs)):
                        nc.vector.tensor_copy(
                            qT_sb[:, mi, pco[j] : pco[j] + pcs[j]], qps[mi, j]
                        )

                # ---- K projection: kT[dk, s] ----
                xk_t = []
                for t in range(KT + 1):
                    pt = 128 if t < KT else 1
                    xt = x_pool.tile([128, S], f16, tag="x", name=f"xk_t{t}")
                    nc.sync.dma_start(
                        out=xt[:pt, :], in_=xk[t * 128 : t * 128 + pt, :]
                    )
                    xk_t.append(xt)
                kps = {}
                for mi in range(MT):
                    for j in range(S // SCH):
                        kps[mi, j] = proj_ps.tile(
                            [128, SCH], f32, tag="ps", name=f"kps_{mi}_{j}"
                        )
                for t in range(KT + 1):
                    pt = 128 if t < KT else 1
                    for mi in range(MT):
                        for j in range(S // SCH):
                            nc.tensor.matmul(
                                kps[mi, j],
                                wk_sb[:pt, t, ts(mi, 128)],
                                xk_t[t][:pt, ts(j, SCH)],
                                start=(t == 0),
                                stop=(t == KT),
                            )
                for mi in range(MT):
                    for j in range(S // SCH):
                        nc.vector.tensor_copy(kT_sb[:, mi, ts(j, SCH)], kps[mi, j])

                # ---- V projection: v[s, dv] (natural layout, K-contiguous) ----
                xv_t = []
                for t in range(KT + 1):
                    pt = 128 if t < KT else 1
                    xt = x_pool.tile([128, S], f16, tag="x", name=f"xv_t{t}")
                    nc.sync.dma_start(
                        out=xt[:pt, :], in_=xv[t * 128 : t * 128 + pt, :]
                    )
                    xv_t.append(xt)
                for si in range(ST):
                    vps = proj_ps.tile([128, DQ], f32, tag="ps", name=f"vps_{si}")
                    for t in range(KT + 1):
                        pt = 128 if t < KT else 1
                        nc.tensor.matmul(
                            vps,
                            xv_t[t][:pt, ts(si, 128)],
                            wv_sb[:pt, t, :],
                            start=(t == 0),
                            stop=(t == KT),
                        )
                    nc.vector.tensor_copy(
                        v_sb[:, si, :, 0:HEAD],
                        vps.rearrange("p (h d) -> p h d", h=NH),
                    )

            # ============ phases B+C: attention + out-projection ============
            with (
                tc.tile_pool(name="exp", bufs=EXP_BUFS) as exp_pool,
                tc.tile_pool(name="nrm", bufs=4) as nrm_pool,
                tc.tile_pool(name="ostage", bufs=2) as out_pool,
                tc.tile_pool(name="sc_ps", bufs=2, space="PSUM") as sc_ps,
                tc.tile_pool(name="ctx_ps", bufs=2, space="PSUM") as ctx_ps,
                tc.tile_pool(name="o_ps", bufs=2, space="PSUM") as o_ps,
            ):
                for j, w in (enumerate(qcs) if 'b' in _PHASES else []):
                    qsl = slice(qco[j], qco[j] + w)
                    # kt group size / scores bank packing for this chunk width
                    if w <= 32:
                        grp, bcol = ST, w        # all 16 kt in one bank
                    elif w <= 256:
                        grp, bcol = 4, 256       # 4 kt across 2 banks
                    else:
                        grp, bcol = 2, 512       # 2 kt across 2 banks
                    for h in range(NH):
                        p0 = HEAD * (h % 2)
                        mt = h // 2
                        cps = ctx_ps.tile([HEAD + 1, w], f32, tag="ctx", name=f"c{j}_{h}")
                        for g in range(ST // grp):
                            scp = sc_ps.tile(
                                [128, grp, bcol], f32, tag="sc", name=f"s{j}_{h}_{g}"
                            )
                            for i in range(grp):
                                nc.tensor.matmul(
                                    scp[:, i, 0:w],
                                    kT_sb[p0 : p0 + HEAD, mt, ts(g * grp + i, 128)],
                                    qT_sb[p0 : p0 + HEAD, mt, qsl],
                                    start=True,
                                    stop=True,
                                )
                            ex = exp_pool.tile(
                                [128, grp, w], f16, tag="exp", name=f"e{j}_{h}_{g}"
                            )
                            nc.scalar.activation(ex, scp[:, :, 0:w], Exp)
                            for i in range(grp):
                                kt = g * grp + i
                                nc.tensor.matmul(
                                    cps,
                                    v_sb[:, kt, h, :],
                                    ex[:, i, :],
                                    start=(kt == 0),
                                    stop=(kt == ST - 1),
                                )
                        rec = nrm_pool.tile([1, w], f32, tag="recip", name=f"r{j}_{h}")
                        nc.vector.reciprocal(rec, cps[HEAD : HEAD + 1, :])
                        bc = nrm_pool.tile([HEAD, w], f32, tag="bc", name=f"b{j}_{h}")
                        nc.gpsimd.partition_broadcast(bc, rec)
                        nc.vector.tensor_mul(
                            ctx_sb[p0 : p0 + HEAD, mt, qsl], cps[0:HEAD, :], bc
                        )

                    if 'c' not in _PHASES:
                        continue
                    # out-projection for this q-chunk
                    ost = out_pool.tile([128, OT, w], f32, tag="ost", name=f"o{j}")
                    for mi in range(OT):
                        op = o_ps.tile([128, w], f32, tag="ops", name=f"op{j}_{mi}")
                        for t2 in range(MT):
                            nc.tensor.matmul(
                                op,
                                wo_sb[:, t2, ts(mi, 128)],
                                ctx_sb[:, t2, qsl],
                                start=(t2 == 0),
                                stop=(t2 == MT - 1),
                            )
                        nc.vector.tensor_copy(ost[:, mi, :], op)
                    nc.sync.dma_start(
                        out=outp.rearrange("(t p) n -> p t n", p=128)[:, :, qsl],
                        in_=ost,
                    )

    nc.finalize()
    return nc


def _get_program(nq):
    if nq not in _cache:
        _cache[nq] = _build(nq)
    return _cache[nq]


def kernel(query, key, value, mask, Wq, bq, Wk, bk, Wv, bv, Wo, bo):
    from concourse.bass_utils import run_bass_kernel_spmd

    query = np.asarray(query, dtype=np.float32)
    key = np.asarray(key, dtype=np.float32)
    value = np.asarray(value, dtype=np.float32)
    mask = np.asarray(mask)
    Wq = np.asarray(Wq, dtype=np.float32)
    bq = np.asarray(bq, dtype=np.float32)
    Wk = np.asarray(Wk, dtype=np.float32)
    bk = np.asarray(bk, dtype=np.float32)
    Wv = np.asarray(Wv, dtype=np.float32)
    bv = np.asarray(bv, dtype=np.float32)
    Wo = np.asarray(Wo, dtype=np.float32)
    bo = np.asarray(bo, dtype=np.float32)

    idxs = [np.nonzero(mask[b] != 0)[0] for b in range(B)]
    packed = all(len(ix) <= NQ_PACKED for ix in idxs)
    if packed:
        nq = NQ_PACKED
    else:
        # Degenerate mask (can't happen for the reference seed): process every
        # query column; masked columns are zeroed + aug-row 0, which yields the
        # exact uniform-softmax rows on device.
        nq = S
        idxs = [np.arange(S) for _ in range(B)]

    scale = 1.0 / np.sqrt(np.float32(HEAD))
    in_maps = []
    for c in range(NCORES):
        b, g = divmod(c, GROUPS)
        rows = slice(DQ * g, DQ * (g + 1))

        ix = idxs[b]
        xq_h = np.zeros((KAUG, nq), np.float32)
        xcols = query[b][ix].T
        if not packed:
            xcols = xcols * (mask[b] != 0)
        xq_h[:HIDDEN, : len(ix)] = xcols
        xq_h[HIDDEN, : len(ix)] = (mask[b][ix] != 0).astype(np.float32)

        xk_h = np.empty((KAUG, S), np.float32)
        xk_h[:HIDDEN] = key[b].T
        xk_h[HIDDEN] = 1.0
        xv_h = np.empty((KAUG, S), np.float32)
        xv_h[:HIDDEN] = value[b].T
        xv_h[HIDDEN] = 1.0

        wq_h = np.empty((KAUG, DQ), np.float32)
        wq_h[:HIDDEN] = Wq[rows].T * scale
        wq_h[HIDDEN] = bq[rows] * scale
        wk_h = np.empty((KAUG, DQ), np.float32)
        wk_h[:HIDDEN] = Wk[rows].T
        wk_h[HIDDEN] = bk[rows]
        wv_h = np.empty((KAUG, DQ), np.float32)
        wv_h[:HIDDEN] = Wv[rows].T
        wv_h[HIDDEN] = bv[rows]
        wo_h = np.ascontiguousarray(Wo[:, rows].T)

        in_maps.append(
            {
                "xq": xq_h.astype(np.float16),
                "xk": xk_h.astype(np.float16),
                "xv": xv_h.astype(np.float16),
                "wq": wq_h.astype(np.float16),
                "wk": wk_h.astype(np.float16),
                "wv": wv_h.astype(np.float16),
                "wo": wo_h.astype(np.float16),
            }
        )

    nc = _get_program(nq)
    res = run_bass_kernel_spmd(nc, in_maps, core_ids=list(range(NCORES)))

    out = np.empty((B, S, HIDDEN), np.float32)
    for b in range(B):
        part = sum(res.results[b * GROUPS + g]["outp"] for g in range(GROUPS))
        ix = idxs[b]
        out[b][ix] = part[:, : len(ix)].T + bo
        if packed:
            # masked rows: softmax is uniform -> mean(V) @ Wo^T + bo, exact.
            vbar = value[b].mean(axis=0) @ Wv.T + bv
            out[b][mask[b] == 0] = vbar @ Wo.T + bo
    return out
